# revision 11
# baseline (speedup 1.0000x reference)
"""Trainium2 Bass kernel for nn_ClassicalSelfAttention.

Reference computation (per token n, E=1024, H=16 heads, D=64):
    q = x @ Wq.T ; k = x @ Wk.T ; v = x @ Wv.T          # [N, E]
    scores[n,h,g] = sum_d q[n,h,d] k[n,g,d] / 8          # per-token head attn
    attn = softmax(scores, axis=g)
    out[n,h,d] = sum_g attn[n,h,g] v[n,g,d]
    y = out.reshape(N,E) @ Wo.T

Sharding: data-parallel over tokens, 8 cores x 8192 tokens, weights
replicated, no cross-core communication.

Per-core design (token tiles of 128 on SBUF partitions):
  - PE (TensorE): x-tile transpose, all four projections as fp32r matmuls
    (full fp32 data, 1 cycle/row), out_attn transpose.
  - The per-token 16x16 head attention cannot be batched on PE (both
    operands vary per token), so it runs token-major on DVE with bf16
    multiplies + fp32-accumulated grouped reduces; exp on ScalarE.
  - Wv's output-feature permutation (e'=d*16+g) is folded into the weight
    load so the AV reduction is over the contiguous innermost axis.
"""

import os
import sys

import numpy as np

sys.path.insert(0, "/opt/trn_rl_repo")

import concourse.bass as bass
import concourse.mybir as mybir
import concourse.tile as tile
from concourse.masks import make_identity

EMBED = 1024
NH = 16
HD = 64
N_TOKENS = 65536
N_CORES = 8
N_SHARD = N_TOKENS // N_CORES
P = 128
KO = EMBED // P  # 8 contraction blocks

F32 = mybir.dt.float32
F32R = mybir.dt.float32r


def _build_body(ctx, tc, x, wq, wk, wv, wo, y, n_shard, attn_dt, use_f32r):
    nc = tc.nc

    def mm_dt(ap):
        return ap.bitcast(F32R) if use_f32r else ap

    wpool = ctx.enter_context(tc.tile_pool(name="wpool", bufs=1))
    work = ctx.enter_context(tc.tile_pool(name="work", bufs=2))
    psum = ctx.enter_context(tc.tile_pool(name="psum", bufs=2, space="PSUM"))

    ident = wpool.tile([P, P], F32)
    make_identity(nc, ident)

    # ---------------- weight prep: WT[ki, ko, e] = W[e, ko*P+ki] ----------
    # Wv's output features are additionally permuted (e' = d*16+g) via the
    # copy access pattern so v comes out of its projection as [n, (d,g)].
    wts = {}
    for name, w in (("q", wq), ("k", wk), ("v", wv), ("o", wo)):
        wt = wpool.tile([P, KO, EMBED], F32, tag=f"wt_{name}")
        wt_v_view = wt.rearrange("p ko (d g) -> p ko g d", d=HD, g=NH)
        for eb in range(KO):
            stg = work.tile([P, EMBED], F32, tag="x")
            nc.gpsimd.dma_start(out=stg, in_=w[eb * P : (eb + 1) * P, :])
            tp = psum.tile([P, KO, P], F32, tag="psA")
            for kb in range(KO):
                nc.tensor.transpose(
                    tp[:, kb, :], stg[:, kb * P : (kb + 1) * P], ident
                )
            if name == "v":
                # PSUM block holds [g_loc(2), d(64)] of e=g*64+d; scatter
                # into the (d,g)-major layout.
                nc.scalar.copy(
                    mm_dt(wt_v_view[:, :, 2 * eb : 2 * eb + 2, :]),
                    tp.rearrange("p ko (g d) -> p ko g d", g=2),
                )
            else:
                nc.scalar.copy(
                    mm_dt(wt[:, :, eb * P : (eb + 1) * P]), tp[:]
                )
        wts[name] = wt

    inv_sqrt_d = 1.0 / float(np.sqrt(HD))
    n_tiles = n_shard // P

    for it in range(n_tiles):
        tok = slice(it * P, (it + 1) * P)

        x_t = work.tile([P, EMBED], F32, tag="x")
        nc.gpsimd.dma_start(out=x_t, in_=x[tok, :])

        # xT[ki, kb, n] = x[n, kb*P+ki]
        xT_ps = psum.tile([P, KO, P], F32, tag="psA")
        for kb in range(KO):
            nc.tensor.transpose(xT_ps[:, kb, :], x_t[:, kb * P : (kb + 1) * P], ident)
        xT = work.tile([P, KO, P], F32, tag="xT")
        nc.scalar.copy(mm_dt(xT[:]), xT_ps)

        # q/k/v projections: [n, e] += xT_kb.T @ WT_kb
        qkv = {}
        for nm in ("q", "k", "v"):
            acc = psum.tile([P, EMBED], F32, tag="psB")
            for kb in range(KO):
                for eh in range(2):
                    nc.tensor.matmul(
                        acc[:, eh * 512 : (eh + 1) * 512],
                        mm_dt(xT[:, kb, :]),
                        mm_dt(wts[nm][:, kb, eh * 512 : (eh + 1) * 512]),
                        start=(kb == 0),
                        stop=(kb == KO - 1),
                    )
            t = work.tile([P, EMBED], attn_dt, tag=f"{nm}_a")
            nc.scalar.copy(t, acc)
            qkv[nm] = t

        q_a, k_a, v_a = qkv["q"], qkv["k"], qkv["v"]
        k3 = k_a.rearrange("p (g d) -> p g d", g=NH)   # [P, g, d]
        v3 = v_a.rearrange("p (d g) -> p d g", d=HD)   # [P, d, g] (permuted Wv)

        # scores[n,h,g] = sum_d q[n,h,d] * k[n,g,d]
        scores = work.tile([P, NH, NH], F32, tag="scores")
        for h in range(NH):
            sc_tmp = work.tile([P, NH, HD], attn_dt, tag="sctmp")
            qh = q_a[:, None, h * HD : (h + 1) * HD].to_broadcast((P, NH, HD))
            nc.vector.tensor_mul(sc_tmp, k3, qh)
            nc.vector.tensor_reduce(
                out=scores[:, h, :],
                in_=sc_tmp,
                axis=mybir.AxisListType.X,
                op=mybir.AluOpType.add,
            )

        # softmax over g (scale folded into exp; no max-shift needed, |s|<~8)
        probs = work.tile([P, NH, NH], attn_dt, tag="probs")
        nc.scalar.activation(
            out=probs, in_=scores, func=mybir.ActivationFunctionType.Exp,
            scale=inv_sqrt_d,
        )
        z = work.tile([P, NH], F32, tag="z")
        nc.vector.tensor_reduce(
            out=z, in_=probs, axis=mybir.AxisListType.X, op=mybir.AluOpType.add
        )
        rz = work.tile([P, NH], F32, tag="rz")
        nc.vector.reciprocal(rz, z)
        pn = work.tile([P, NH, NH], attn_dt, tag="pn")
        nc.vector.tensor_mul(pn, probs, rz[:, :, None].to_broadcast((P, NH, NH)))

        # out_attn[n,h,d] = sum_g pn[n,h,g] * v[n,d,g]
        out_attn = work.tile([P, NH, HD], F32, tag="oat")
        for h in range(NH):
            av_tmp = work.tile([P, HD, NH], attn_dt, tag="avtmp")
            ph = pn[:, h, :][:, None, :].to_broadcast((P, HD, NH))
            nc.vector.tensor_mul(av_tmp, v3, ph)
            nc.vector.tensor_reduce(
                out=out_attn[:, h, :],
                in_=av_tmp,
                axis=mybir.AxisListType.X,
                op=mybir.AluOpType.add,
            )

        # y = out_attn_flat @ Wo.T
        oa_flat = out_attn.rearrange("p h d -> p (h d)")
        oaT_ps = psum.tile([P, KO, P], F32, tag="psA")
        for kb in range(KO):
            nc.tensor.transpose(
                oaT_ps[:, kb, :], oa_flat[:, kb * P : (kb + 1) * P], ident
            )
        oaT = work.tile([P, KO, P], F32, tag="oaT")
        nc.scalar.copy(mm_dt(oaT[:]), oaT_ps)

        yacc = psum.tile([P, EMBED], F32, tag="psB")
        for kb in range(KO):
            for eh in range(2):
                nc.tensor.matmul(
                    yacc[:, eh * 512 : (eh + 1) * 512],
                    mm_dt(oaT[:, kb, :]),
                    mm_dt(wts["o"][:, kb, eh * 512 : (eh + 1) * 512]),
                    start=(kb == 0),
                    stop=(kb == KO - 1),
                )
        y_t = work.tile([P, EMBED], F32, tag="y")
        nc.scalar.copy(y_t, yacc)
        nc.sync.dma_start(out=y[tok, :], in_=y_t)


def build_kernel(n_shard=N_SHARD, attn_dt=mybir.dt.bfloat16, use_f32r=True):
    from concourse import bacc

    # Bacc (not raw Bass): its finalize() runs generate_event_semaphores,
    # which splits multi-sem waits — walrus allows only 1 wait/instruction.
    nc = bacc.Bacc()
    x = nc.dram_tensor("x", [n_shard, EMBED], F32, kind="ExternalInput")
    wq = nc.dram_tensor("Wq", [EMBED, EMBED], F32, kind="ExternalInput")
    wk = nc.dram_tensor("Wk", [EMBED, EMBED], F32, kind="ExternalInput")
    wv = nc.dram_tensor("Wv", [EMBED, EMBED], F32, kind="ExternalInput")
    wo = nc.dram_tensor("Wo", [EMBED, EMBED], F32, kind="ExternalInput")
    y = nc.dram_tensor("y", [n_shard, EMBED], F32, kind="ExternalOutput")
    from contextlib import ExitStack

    with tile.TileContext(nc) as tc, ExitStack() as ctx:
        _build_body(ctx, tc, x[:], wq[:], wk[:], wv[:], wo[:], y[:], n_shard, attn_dt, use_f32r)
    if not nc.is_finalized():
        nc.finalize()
    return nc


_NC_CACHE = {}


def kernel(x, Wq, Wk, Wv, Wo):
    """Full-input entry point: shards tokens over 8 cores, runs, gathers."""
    from concourse.bass_utils import run_bass_kernel_spmd

    x = np.ascontiguousarray(np.asarray(x, dtype=np.float32))
    Wq = np.ascontiguousarray(np.asarray(Wq, dtype=np.float32))
    Wk = np.ascontiguousarray(np.asarray(Wk, dtype=np.float32))
    Wv = np.ascontiguousarray(np.asarray(Wv, dtype=np.float32))
    Wo = np.ascontiguousarray(np.asarray(Wo, dtype=np.float32))

    n = x.shape[0]
    n_shard = n // N_CORES
    key = n_shard
    if key not in _NC_CACHE:
        _NC_CACHE[key] = build_kernel(n_shard=n_shard)
    nc = _NC_CACHE[key]

    in_maps = [
        {
            "x": x[i * n_shard : (i + 1) * n_shard],
            "Wq": Wq,
            "Wk": Wk,
            "Wv": Wv,
            "Wo": Wo,
        }
        for i in range(N_CORES)
    ]
    res = run_bass_kernel_spmd(nc, in_maps, list(range(N_CORES)))
    kernel._last_exec_ns = res.exec_time_ns
    out = np.concatenate([res.results[i]["y"] for i in range(N_CORES)], axis=0)
    return out
